# revision 15
# baseline (speedup 1.0000x reference)
"""PerResidueLDDTHead kernel for 8x TRN2 NeuronCores.

Math: logits = onehot @ s @ W + b  ==  onehot @ (s @ W) + b
  stage 1 (on device): y = s @ W          [n_res, c_out]   (tiny matmul)
  stage 2 (on device): outT = y.T @ ohT   [c_out, n_atom/8] (streams the
          one-hot shard, the memory-bound part) ; + b fused into the
          PSUM->SBUF copy as a per-partition scalar add.

Sharding: n_atom split across the 8 cores; s/W/b replicated.
Host only reshapes/transposes for layout (no FLOPs on host).
"""

import os
import numpy as np

import concourse.bass as bass
import concourse.tile as tile
from concourse import bacc, mybir
from concourse.bass_utils import run_bass_kernel_spmd

N_RES = 2048
N_ATOM = 32768
C_S = 384
C_OUT = 50
N_CORES = 8
APC = N_ATOM // N_CORES  # atoms per core

F32 = mybir.dt.float32
P = 128      # partition size
AG = 512     # atom columns per PSUM bank (fp32 moving max / bank size)


def build(n_res=N_RES, apc=APC, c_s=C_S, c_out=C_OUT, oh_bufs=3, repeat=1,
          debug=False):
    # residue chunks: (start, width), width <= P, last may be partial
    chunks = []
    r0 = 0
    while r0 < n_res:
        chunks.append((r0, min(P, n_res - r0)))
        r0 += P
    rc_n = len(chunks)
    cc_n = c_s // P       # channel chunks
    ag_n = apc // AG      # atom groups (PSUM banks), must be <= 8
    assert ag_n <= 8

    nc = bacc.Bacc(
        "TRN2", target_bir_lowering=False, debug=debug, num_devices=N_CORES
    )

    ohT = nc.dram_tensor("ohT", [n_res, apc], F32, kind="ExternalInput").ap()
    sT = nc.dram_tensor("sT", [c_s, n_res], F32, kind="ExternalInput").ap()
    Wd = nc.dram_tensor("W", [c_s, c_out], F32, kind="ExternalInput").ap()
    bd = nc.dram_tensor("b", [c_out, 1], F32, kind="ExternalInput").ap()
    outT = nc.dram_tensor("outT", [c_out, apc], F32, kind="ExternalOutput").ap()

    with tile.TileContext(nc) as tc:
        for rep in range(repeat):
            with (
                tc.tile_pool(name=f"const{rep}", bufs=1) as const,
                tc.tile_pool(name=f"ohp{rep}", bufs=oh_bufs) as ohp,
            ):
                sT_t = []
                for cc in range(cc_n):
                    t = const.tile([P, n_res], F32, tag=f"sT{cc}",
                                   name=f"sT{rep}_{cc}")
                    nc.sync.dma_start(t[:], sT[cc * P:(cc + 1) * P, :])
                    sT_t.append(t)
                W_t = []
                for cc in range(cc_n):
                    t = const.tile([P, c_out], F32, tag=f"W{cc}",
                                   name=f"W{rep}_{cc}")
                    nc.sync.dma_start(t[:], Wd[cc * P:(cc + 1) * P, :])
                    W_t.append(t)
                b_t = const.tile([c_out, 1], F32, tag="b", name=f"b{rep}")
                nc.sync.dma_start(b_t[:], bd[:])

                # stage 1: y[r, o] = sum_c s[r, c] W[c, o], computed per
                # residue chunk: psum = sT_chunk.T @ W_chunk
                y_t = []
                with tc.tile_pool(
                    name=f"psum_y{rep}", bufs=2, space=bass.MemorySpace.PSUM
                ) as psy:
                    for rc, (r0, rw) in enumerate(chunks):
                        py = psy.tile([rw, c_out], F32, tag="py",
                                      name=f"py{rep}_{rc}")
                        for cc in range(cc_n):
                            nc.tensor.matmul(
                                py[:],
                                sT_t[cc][:, r0:r0 + rw],
                                W_t[cc][:],
                                start=(cc == 0),
                                stop=(cc == cc_n - 1),
                            )
                        yt = const.tile([rw, c_out], F32, tag=f"y{rc}",
                                        name=f"y{rep}_{rc}")
                        nc.vector.tensor_copy(yt[:], py[:])
                        y_t.append(yt)

                # stage 2: outT[o, a] = sum_r y[r, o] * ohT[r, a] (+ b[o])
                # Last chunk's DMA is split per atom-group so each group's
                # closing matmul + bias-copy + store overlaps the stream.
                out_sb = const.tile([c_out, apc], F32, tag="out",
                                    name=f"out_sb{rep}")
                with tc.tile_pool(
                    name=f"psum_o{rep}", bufs=1, space=bass.MemorySpace.PSUM
                ) as pso:
                    ps = [
                        pso.tile([c_out, AG], F32, tag=f"po{ag}",
                                 name=f"po{rep}_{ag}")
                        for ag in range(ag_n)
                    ]
                    for rc, (r0, rw) in enumerate(chunks[:-1]):
                        oh_t = ohp.tile([rw, apc], F32, tag="oh",
                                        name=f"oh{rep}_{rc}")
                        nc.sync.dma_start(oh_t[:], ohT[r0:r0 + rw, :])
                        for ag in range(ag_n):
                            nc.tensor.matmul(
                                ps[ag][:],
                                y_t[rc][:],
                                oh_t[:, ag * AG:(ag + 1) * AG],
                                start=(rc == 0),
                                stop=False,
                            )
                    r0, rw = chunks[-1]
                    for ag in range(ag_n):
                        a0 = ag * AG
                        ohl = ohp.tile([rw, AG], F32, tag=f"ohl{ag}",
                                       name=f"ohl{rep}_{ag}")
                        nc.sync.dma_start(
                            ohl[:], ohT[r0:r0 + rw, a0:a0 + AG]
                        )
                        nc.tensor.matmul(
                            ps[ag][:],
                            y_t[rc_n - 1][:],
                            ohl[:],
                            start=(rc_n == 1),
                            stop=True,
                        )
                        nc.vector.tensor_scalar_add(
                            out_sb[:, a0:a0 + AG], ps[ag][:], b_t[:]
                        )
                        nc.sync.dma_start(
                            outT[:, a0:a0 + AG], out_sb[:, a0:a0 + AG]
                        )

    nc.compile()
    return nc


BAND = 288  # fixed residue-band width for the sliced fast path
GW = 64     # residue window per atom group (grouped fast path)
GA = 512    # atoms per group (one PSUM bank of fp32 columns)
NG = APC // GA  # groups per core
U8 = mybir.dt.uint8


def build_grouped(n_groups=NG, gw=GW, ga=GA, c_s=C_S, c_out=C_OUT, repeat=1,
                  oh_bufs=3, debug=False):
    """Grouped fast path: atoms are sorted, so each 512-atom group touches a
    <=GW-wide residue window. Host slices per-group windows of oh (as u8) and
    sT; stage 2 is ONE matmul per group (4096 cols total vs 12288 banded).

    Inputs per core:
      ohg [n_groups*gw, ga] u8 : group g rows g*gw..  = oh[atoms_g, win_g].T
      sTg [c_s, n_groups*gw]   : group g cols = s[win_g, :].T
      W [c_s, c_out], b [c_out, 1]
    Output: outT [c_out, n_groups*ga].
    """
    cc_n = c_s // P
    # stage-1 quarters: y for 128 window-residues at a time; group g's lhsT
    # is a partition-offset slice of quarter q = g // gpq
    gpq = P // gw          # groups per quarter
    q_n = n_groups // gpq  # quarters
    assert n_groups % gpq == 0

    nc = bacc.Bacc(
        "TRN2", target_bir_lowering=False, debug=debug, num_devices=N_CORES
    )
    ohg = nc.dram_tensor("ohg", [n_groups * gw, ga], U8,
                         kind="ExternalInput").ap()
    sTg = nc.dram_tensor("sTg", [c_s, n_groups * gw], F32,
                         kind="ExternalInput").ap()
    Wd = nc.dram_tensor("W", [c_s, c_out], F32, kind="ExternalInput").ap()
    bd = nc.dram_tensor("b", [c_out, 1], F32, kind="ExternalInput").ap()
    outT = nc.dram_tensor("outT", [c_out, n_groups * ga], F32,
                          kind="ExternalOutput").ap()

    with tile.TileContext(nc) as tc:
        for rep in range(repeat):
            with (
                tc.tile_pool(name=f"gconst{rep}", bufs=1) as const,
                tc.tile_pool(name=f"gohp{rep}", bufs=oh_bufs) as ohp,
                tc.tile_pool(name=f"gpsy{rep}", bufs=2,
                             space=bass.MemorySpace.PSUM) as psy,
                tc.tile_pool(name=f"gpso{rep}", bufs=3,
                             space=bass.MemorySpace.PSUM) as pso,
            ):
                W_t = []
                for cc in range(cc_n):
                    t = const.tile([P, c_out], F32, tag=f"W{cc}",
                                   name=f"gW{rep}_{cc}")
                    nc.sync.dma_start(t[:], Wd[cc * P:(cc + 1) * P, :])
                    W_t.append(t)
                b_t = const.tile([c_out, 1], F32, tag="b", name=f"gb{rep}")
                nc.sync.dma_start(b_t[:], bd[:])
                out_sb = const.tile([c_out, n_groups * ga], F32, tag="out",
                                    name=f"gout_sb{rep}")

                # sTg quarters [P, P] so stage 1 starts after 3 small DMAs
                sq_t = {}
                for q in range(q_n):
                    for cc in range(cc_n):
                        t = const.tile([P, P], F32, tag=f"sq{q}_{cc}",
                                       name=f"gsq{rep}_{q}_{cc}")
                        nc.sync.dma_start(
                            t[:],
                            sTg[cc * P:(cc + 1) * P, q * P:(q + 1) * P],
                        )
                        sq_t[q, cc] = t

                # oh windows: u8 DRAM -> f32 SBUF cast-DMA on gpsimd (SWDGE)
                oh_t = []
                for g in range(n_groups):
                    t = ohp.tile([gw, ga], F32, tag="oh", name=f"goh{rep}_{g}")
                    nc.gpsimd.dma_start(t[:], ohg[g * gw:(g + 1) * gw, :])
                    oh_t.append(t)

                for g in range(n_groups):
                    q, w0 = g // gpq, (g % gpq) * gw
                    a0 = g * ga
                    py = psy.tile([gw, c_out], F32, tag="py",
                                  name=f"gpy{rep}_{g}")
                    for cc in range(cc_n):
                        nc.tensor.matmul(
                            py[:], sq_t[q, cc][:, w0:w0 + gw], W_t[cc][:],
                            start=(cc == 0), stop=(cc == cc_n - 1),
                        )
                    yg = const.tile([gw, c_out], F32, tag=f"y{g}",
                                    name=f"gy{rep}_{g}")
                    nc.scalar.copy(yg[:], py[:])
                    po = pso.tile([c_out, ga], F32, tag="po",
                                  name=f"gpo{rep}_{g}")
                    nc.tensor.matmul(
                        po[:], yg[:], oh_t[g][:], start=True, stop=True,
                    )
                    if g % 2 == 0:
                        nc.vector.tensor_scalar_add(
                            out_sb[:, a0:a0 + ga], po[:], b_t[:]
                        )
                    else:
                        nc.scalar.add(
                            out_sb[:, a0:a0 + ga], po[:], b_t[:]
                        )
                    nc.scalar.dma_start(
                        outT[:, a0:a0 + ga], out_sb[:, a0:a0 + ga]
                    )

    nc.compile()
    return nc


_NC_CACHE = {}


def _get_nc(n_res=N_RES, repeat=1):
    key = (n_res, repeat)
    if key not in _NC_CACHE:
        _NC_CACHE[key] = build(n_res=n_res, repeat=repeat)
    return _NC_CACHE[key]


def _get_grouped_nc(repeat=1):
    key = ("grouped", repeat)
    if key not in _NC_CACHE:
        _NC_CACHE[key] = build_grouped(repeat=repeat)
    return _NC_CACHE[key]


def detect_groups(oh):
    """Per-(core, group) start of a GW-wide residue window covering all
    nonzeros of that 512-atom block, with values verified to be exact 0/1
    inside the window; None if any block doesn't fit (band/full fallback)."""
    gstarts = []
    for m in range(N_CORES):
        row = []
        for g in range(NG):
            blk = oh[m * APC + g * GA: m * APC + (g + 1) * GA]
            nz = np.flatnonzero(blk.any(axis=0))
            if len(nz) == 0:
                row.append(0)
                continue
            lo, hi = int(nz[0]), int(nz[-1])
            if hi - lo + 1 > GW:
                return None
            st = min(lo, N_RES - GW)
            win = blk[:, st:st + GW]
            if not np.array_equal(win, win.astype(np.uint8)):
                return None
            row.append(st)
        gstarts.append(row)
    return gstarts


def prep_group_in_maps(s, oh, W, b, gstarts):
    sT = np.ascontiguousarray(s.T)
    in_maps = []
    for m in range(N_CORES):
        ohg = np.empty((NG * GW, GA), dtype=np.uint8)
        sTg = np.empty((C_S, NG * GW), dtype=np.float32)
        for g, st in enumerate(gstarts[m]):
            blk = oh[m * APC + g * GA: m * APC + (g + 1) * GA, st:st + GW]
            ohg[g * GW:(g + 1) * GW] = blk.T
            sTg[:, g * GW:(g + 1) * GW] = sT[:, st:st + GW]
        in_maps.append({"ohg": ohg, "sTg": sTg, "W": W, "b": b})
    return in_maps


def detect_bands(oh):
    """Per-core start of a BAND-wide residue window covering all nonzero
    rows of that core's ohT shard; None if any shard doesn't fit (then
    the full-width kernel is used). Exact for any input."""
    starts = []
    for m in range(N_CORES):
        shard = oh[m * APC:(m + 1) * APC]
        nz = np.flatnonzero(shard.any(axis=0))
        if len(nz) == 0:
            starts.append(0)
            continue
        lo, hi = int(nz[0]), int(nz[-1])
        if hi - lo + 1 > BAND:
            return None
        starts.append(min(lo, N_RES - BAND))
    return starts


def prep_in_maps(s, oh, W, b):
    sT = np.ascontiguousarray(s.T)
    in_maps = []
    for m in range(N_CORES):
        ohT_m = np.ascontiguousarray(oh[m * APC:(m + 1) * APC, :].T)
        in_maps.append({"ohT": ohT_m, "sT": sT, "W": W, "b": b})
    return in_maps


def prep_band_in_maps(s, oh, W, b, starts):
    in_maps = []
    for m, st in enumerate(starts):
        ohT_m = np.ascontiguousarray(oh[m * APC:(m + 1) * APC, st:st + BAND].T)
        sT_m = np.ascontiguousarray(s[st:st + BAND, :].T)
        in_maps.append({"ohT": ohT_m, "sT": sT_m, "W": W, "b": b})
    return in_maps


def _cast_inputs(s, token_to_atom_idx, W, b):
    s = np.ascontiguousarray(np.asarray(s, dtype=np.float32))
    oh = np.asarray(token_to_atom_idx, dtype=np.float32)
    W = np.ascontiguousarray(np.asarray(W, dtype=np.float32))
    b = np.ascontiguousarray(np.asarray(b, dtype=np.float32).reshape(C_OUT, 1))
    return s, oh, W, b


def assemble_out(results):
    out = np.empty((N_ATOM, C_OUT), dtype=np.float32)
    for m, r in enumerate(results):
        out[m * APC:(m + 1) * APC, :] = r["outT"].T
    return out


def kernel_with_results(s, token_to_atom_idx, W, b, trace=False):
    s, oh, W, b = _cast_inputs(s, token_to_atom_idx, W, b)
    gstarts = detect_groups(oh)
    if gstarts is not None:
        nc = _get_grouped_nc()
        in_maps = prep_group_in_maps(s, oh, W, b, gstarts)
    else:
        starts = detect_bands(oh)
        if starts is not None:
            nc = _get_nc(BAND)
            in_maps = prep_band_in_maps(s, oh, W, b, starts)
        else:
            nc = _get_nc(N_RES)
            in_maps = prep_in_maps(s, oh, W, b)
    res = run_bass_kernel_spmd(nc, in_maps, list(range(N_CORES)), trace=trace)
    return assemble_out(res.results), res


def kernel(s, token_to_atom_idx, W, b):
    trace = bool(int(os.environ.get("KERNEL_TRACE", "0")))
    out, _ = kernel_with_results(s, token_to_atom_idx, W, b, trace=trace)
    return out
